# revision 1
# baseline (speedup 1.0000x reference)
import numpy as np
import jax
import jax.numpy as jnp

# Problem constants (hardcoded; kernel.py must be self-contained).
ID, HD, B, T = 128, 256, 512, 128
MAX_REAL = 64
CAT_SEGS = [(64, 72), (72, 88), (88, 128)]
OUT_DIM = 128

_W_NAMES = (
    "Wi_f", "bi_f", "Wh_f", "bh_f",
    "Wi_r", "bi_r", "Wh_r", "bh_r",
    "W1", "b1", "W2", "b2", "W3", "b3",
)


def _lstm_scan(x, Wi, bi, Wh, bh):
    b = x.shape[0]
    hd = Wh.shape[0]

    def step(carry, xt):
        h, c = carry
        gates = xt @ Wi + bi + h @ Wh + bh
        f, i, a, o = jnp.split(gates, 4, axis=-1)
        f = jax.nn.sigmoid(f)
        i = jax.nn.sigmoid(i)
        a = jnp.tanh(a)
        o = jax.nn.sigmoid(o)
        c = f * c + i * a
        h = o * jnp.tanh(c)
        return (h, c), h

    init = (jnp.zeros((b, hd), x.dtype), jnp.zeros((b, hd), x.dtype))
    _, hs = jax.lax.scan(step, init, jnp.swapaxes(x, 0, 1))
    return jnp.swapaxes(hs, 0, 1)


def _forward(x0, Wi_f, bi_f, Wh_f, bh_f, Wi_r, bi_r, Wh_r, bh_r,
             W1, b1, W2, b2, W3, b3):
    # The input-projection x0 @ Wi has no recurrent dependency: hoist it out
    # of the scan by folding it into the scanned input so the sequential part
    # only does the [b, HD] @ [HD, 4HD] recurrent matmul per step.
    xi_f = jnp.einsum("btd,dg->btg", x0, Wi_f) + bi_f + bh_f
    xi_r = jnp.einsum("btd,dg->btg", x0[:, ::-1, :], Wi_r) + bi_r + bh_r

    def make_step(Wh):
        def step(carry, gt):
            h, c = carry
            gates = gt + h @ Wh
            f, i, a, o = jnp.split(gates, 4, axis=-1)
            f = jax.nn.sigmoid(f)
            i = jax.nn.sigmoid(i)
            a = jnp.tanh(a)
            o = jax.nn.sigmoid(o)
            c = f * c + i * a
            h = o * jnp.tanh(c)
            return (h, c), h
        return step

    b = x0.shape[0]
    init = (jnp.zeros((b, HD), x0.dtype), jnp.zeros((b, HD), x0.dtype))
    _, hs_f = jax.lax.scan(make_step(Wh_f), init, jnp.swapaxes(xi_f, 0, 1))
    _, hs_r = jax.lax.scan(make_step(Wh_r), init, jnp.swapaxes(xi_r, 0, 1))
    x1 = jnp.concatenate([jnp.swapaxes(hs_f, 0, 1), jnp.swapaxes(hs_r, 0, 1)], axis=2)

    x2 = jax.nn.leaky_relu(x1 @ W1 + b1, 0.1)
    x3 = jax.nn.leaky_relu(x2 @ W2 + b2, 0.1)
    x4 = x3 @ W3 + b3
    parts = [jax.nn.sigmoid(x4[..., :MAX_REAL])]
    for s0, s1 in CAT_SEGS:
        parts.append(jax.nn.softmax(x4[..., s0:s1], axis=-1))
    return jnp.concatenate(parts, axis=-1)


_PMAP_CACHE = {}


def _get_fn(nd):
    if nd not in _PMAP_CACHE:
        if nd > 1:
            _PMAP_CACHE[nd] = jax.pmap(_forward, in_axes=(0,) + (None,) * 14)
        else:
            _PMAP_CACHE[nd] = jax.jit(_forward)
    return _PMAP_CACHE[nd]


def kernel(**inputs):
    x0 = np.asarray(inputs["x0"], np.float32)
    ws = [np.asarray(inputs[k], np.float32) for k in _W_NAMES]

    n = jax.device_count()
    nd = 1
    for d in (8, 4, 2):
        if d <= n and B % d == 0:
            nd = d
            break

    try:
        if nd > 1:
            fn = _get_fn(nd)
            xs = x0.reshape(nd, B // nd, T, ID)
            out = np.asarray(fn(xs, *ws)).reshape(B, T, OUT_DIM)
        else:
            out = np.asarray(_get_fn(1)(x0, *ws))
    except Exception:
        out = np.asarray(_get_fn(1)(x0, *ws))
    return out.astype(np.float32)
